# revision 54
# baseline (speedup 1.0000x reference)
"""Sparse (range-masked) GQA attention on 8 Trainium2 NeuronCores.

Strategy: tensor-parallel over heads. Core c owns q heads {2c, 2c+1} and kv
head c//2, so each core runs an independent single-core attention over the
full sequence — no collectives; the host concatenates the 8 head-slices.

Per-core kernel (Bass/Tile), HW-calibrated design points:
  1. Inputs staged bf16 on the host (halves HBM traffic; arithmetic stays on
     device); all loads ride SP's HWDGE queue (gpsimd DMA is software-DGE,
     ~us/trigger), halved and ordered k/cos/sin -> q -> v per half.
  2. RoPE + K^T/Q^T strip transposes are emitted JUST-IN-TIME per 512-token
     strip, right before the first q-segment that reads the strip: RoPE (all
     on DVE, bf16 2x modes), 4 packed PE transposes into a ps-ring PSUM
     tile, one DVE copy to SBUF. Engines execute in order, so a monolithic
     prologue would queue the next rep's rope behind this rep's tail; JIT
     emission shrinks the rep boundary to one strip chain. Strips, rope
     outputs and V are parity-buffered across reps.
  3. Flat software pipeline over (head, 256-row q segment, group of GRP=4
     active 128-wide k chunks):
       S^T[tk,tq] = KT_chunk.T @ QT_seg          (PE, bf16, PSUM fp32,
                                                  3-deep 2-bank ring)
       P^T = Exp(scale*S^T - 4)                  (ACT, one [128,<=1024] instr
                                                  per group; ACT is the
                                                  bottleneck: ~0.35ns/col +
                                                  ~350ns/instr on HW)
       irregular partial tiles only: P^T *= mask01 (affine_select-built masks)
       out[tq,0:129] += P^T_block.T @ [V | 1]    (PE; ones column => row sums)
     PV matmuls are emitted TWO groups late so PE lookahead absorbs the
     segment-boundary epilogue latency (single-buffered PV banks).
  4. out = PV / max(rowsum, tiny) staged bf16 -> one SP DMA per segment;
     host upcasts to fp32. Uncovered q rows come out as exactly 0.

The tile schedule (skip / dense / partial+rectangles) is computed on the host
from q_ranges/k_ranges — scheduling metadata only; all arithmetic on q/k/v
runs on device. The compiled NEFF is cached per schedule signature.
"""

import math

import numpy as np

T, HQ, HKV, D = 2048, 16, 4, 128
N_CORES = 8
HPC = HQ // N_CORES          # q heads per core
SEG = 512                    # strip width (transposed Q/K layout granularity)
QSEG = 256                   # main-loop q segment: 2 PV banks -> wider exps
GRP = 4                      # k chunks per PSUM score tile / exp instruction
CK = 128                     # tk chunk (contraction tile)
NSEG = T // SEG
NQSEG = T // QSEG
NCK = T // CK
NT = T // 128                # 128-row t-tiles
HALF = D // 2
SCALE = 1.0 / math.sqrt(D)
EXP_BIAS = -4.0              # constant shift; cancels in softmax normalization
SUM_EPS = 1e-30
MAX_PREBUILT_MASKS = 48      # SBUF budget cap; beyond this, build inline

# "pe": PE transpose + ACT/DVE copies (fastest measured — each crossbar
# transpose costs ~2us of DMA-ucode overhead, so "dma"/"sbuf" lose badly);
# "dma": DRAM scratch + crossbar;  "sbuf": direct SBUF->SBUF crossbar
TRANSPOSE_MODE = "pe"

PROFILE = False
LAST_EXEC_NS = None
LAST_RESULT = None

_NEFF_CACHE = {}


def _build_schedule(q_ranges, k_ranges):
    """Per (qseg, kchunk) tile: absent=skip, None=dense, list=mask rectangles.

    Rectangles are in device-tile coordinates: (klo, khi) along the partition
    (tk) axis, (qlo, qhi) along the free (tq) axis, clipped to the tile.
    """
    q_ranges = np.asarray(q_ranges, dtype=np.int64)
    k_ranges = np.asarray(k_ranges, dtype=np.int64)
    idx = np.arange(T)
    qm = (idx[None, :] >= q_ranges[:, :1]) & (idx[None, :] < q_ranges[:, 1:])
    km = (idx[None, :] >= k_ranges[:, :1]) & (idx[None, :] < k_ranges[:, 1:])
    mask = np.zeros((T, T), dtype=bool)
    for r in range(len(q_ranges)):
        if qm[r].any() and km[r].any():
            mask[np.ix_(qm[r], km[r])] = True
    sched = {}
    for s in range(NQSEG):
        for c in range(NCK):
            sub = mask[s * QSEG:(s + 1) * QSEG, c * CK:(c + 1) * CK]
            if not sub.any():
                continue
            if sub.all():
                sched[(s, c)] = None
                continue
            rects = []
            for r in range(len(q_ranges)):
                qlo = max(int(q_ranges[r, 0]) - s * QSEG, 0)
                qhi = min(int(q_ranges[r, 1]) - s * QSEG, QSEG)
                klo = max(int(k_ranges[r, 0]) - c * CK, 0)
                khi = min(int(k_ranges[r, 1]) - c * CK, CK)
                if qlo < qhi and klo < khi:
                    rects.append((klo, khi, qlo, qhi))
            sched[(s, c)] = rects
    return sched


def _sched_key(sched):
    return tuple(sorted(
        (k, -1) if v is None else (k, tuple(v)) for k, v in sched.items()
    ))


def _rect_mask(nc, AO, m, rect, first):
    """AND-chain of affine_select half-planes building a rect indicator."""
    klo, khi, qlo, qhi = rect
    if first:
        nc.gpsimd.memset(m, 1.0)
    if klo > 0:
        nc.gpsimd.affine_select(
            out=m, in_=m, compare_op=AO.is_ge, fill=0.0,
            base=-klo, pattern=[[0, QSEG]], channel_multiplier=1)
    if khi < CK:
        # walrus codegen lacks is_lt: p < khi  <=>  khi - p > 0
        nc.gpsimd.affine_select(
            out=m, in_=m, compare_op=AO.is_gt, fill=0.0,
            base=khi, pattern=[[0, QSEG]], channel_multiplier=-1)
    if qlo > 0:
        nc.gpsimd.affine_select(
            out=m, in_=m, compare_op=AO.is_ge, fill=0.0,
            base=-qlo, pattern=[[1, QSEG]], channel_multiplier=0)
    if qhi < QSEG:
        nc.gpsimd.affine_select(
            out=m, in_=m, compare_op=AO.is_gt, fill=0.0,
            base=qhi, pattern=[[-1, QSEG]], channel_multiplier=0)


def _build_mask_tile(nc, AO, pool, m01_tag, rects, bf16):
    m01 = pool.tile([128, QSEG], bf16, tag=m01_tag, name=m01_tag)
    if len(rects) == 1:
        _rect_mask(nc, AO, m01[:], rects[0], first=True)
    else:
        acc = pool.tile([128, QSEG], bf16, tag=m01_tag + "a", name=m01_tag + "a")
        _rect_mask(nc, AO, m01[:], rects[0], first=True)
        for rect in rects[1:]:
            _rect_mask(nc, AO, acc[:], rect, first=True)
            nc.vector.tensor_tensor(m01[:], m01[:], acc[:], AO.max)
    return m01


def _emit_prologue(nc, pools, ext, rep):
    """Loads + constants + vaug + RoPE for one rep. For rep r+1 this is
    invoked MID-way through rep r's stream, so in-order engines execute it
    under rep r's tail instead of serializing it at the rep boundary."""
    from concourse import mybir

    f32 = mybir.dt.float32
    bf16 = mybir.dt.bfloat16
    AO = mybir.AluOpType
    big = pools[0]
    q_ext, k_ext, v_ext, cos_ext, sin_ext, out_ext = ext

    def btile(shape, dtype, tag):
        return big.tile(shape, dtype, tag=tag, name=f"{tag}_r{rep}")

    # -------- loads (k/cos/sin first, halved, so RoPE starts early) --------
    # ALL inputs ride the sync (SP HWDGE) queue.
    GT = NT // 2          # t-tiles per DMA half
    cos_sb = btile([128, NT, HALF], bf16, "cos_sb")
    sin_sb = btile([128, NT, HALF], bf16, "sin_sb")
    k_sb = btile([128, NT, D], bf16, "k_sb")
    q_sb = btile([128, NT, HPC, D], bf16, "q_sb")
    v_sb = btile([128, NT, D], bf16, "v_sb")
    def load_half(g):
        a, b = g * GT, (g + 1) * GT
        tt = slice(a * 128, b * 128)
        nt = slice(a, b)
        nc.sync.dma_start(
            k_sb[:, nt, :],
            k_ext[tt, :, :].rearrange("(n p) h d -> p n (h d)", p=128))
        nc.sync.dma_start(
            cos_sb[:, nt, :],
            cos_ext[tt, :].rearrange("(n p) d -> p n d", p=128))
        nc.sync.dma_start(
            sin_sb[:, nt, :],
            sin_ext[tt, :].rearrange("(n p) d -> p n d", p=128))
        # q both heads at once: contiguous 1 KiB rows per descriptor
        nc.sync.dma_start(
            q_sb[:, nt, :, :],
            q_ext[tt, :, :].rearrange("(n p) h d -> p n h d", p=128))
        nc.sync.dma_start(
            v_sb[:, nt, :],
            v_ext[tt, :, :].rearrange("(n p) h d -> p n (h d)", p=128))

    load_half(0)
    load_half(1)

    # constants (same values every rep; rebuilds are dep-tracked and cheap)
    bias_sb = btile([128, 1], f32, "bias")
    nc.vector.memset(bias_sb[:], EXP_BIAS)
    zero_seg = btile([128, 2, D], bf16, "zero")
    nc.vector.memset(zero_seg[:], 0.0)

    # V with an appended ones column: [tk, (V | 1)] in bf16. Parity-buffered
    # across reps (and copied in halves) so this rep's copy never waits on
    # the previous rep's last PV reads.
    par = rep % 2
    vaug = btile([128, NT, D + 1], bf16, f"vaug_p{par}")
    nc.gpsimd.memset(vaug[:, :, D:D + 1], 1.0)
    nc.gpsimd.tensor_copy(vaug[:, 0:GT, 0:D], v_sb[:, 0:GT, :])
    nc.gpsimd.tensor_copy(vaug[:, GT:NT, 0:D], v_sb[:, GT:NT, :])

    # strips / rope outputs parity-buffered across reps: this rep's writes
    # never WAR against the previous rep's main-loop reads
    kts = [btile([128, SEG], bf16, f"kts{g}_p{par}") for g in range(NSEG)]
    qts = [[btile([128, SEG], bf16, f"qts{h}_{g}_p{par}") for g in range(NSEG)]
           for h in range(HPC)]
    plans = [(k_sb[:, :, :], kts)] + [
        (q_sb[:, :, h, :], qts[h]) for h in range(HPC)]
    dsts = [btile([128, NT, D], bf16, f"rope{i}_p{par}")
            for i in range(len(plans))]
    tmps = [[btile([128, NT, HALF], bf16, f"t{j}_{i}") for j in range(4)]
            for i in range(len(plans))]
    ident = btile([128, 128], bf16, "ident")
    nc.gpsimd.memset(ident[:], 0.0)
    nc.gpsimd.affine_select(
        out=ident[:], in_=ident[:], compare_op=AO.not_equal, fill=1.0,
        base=0, pattern=[[-1, 128]], channel_multiplier=1)

    # RoPE, whole-tensor (fewest/widest ops -- per-instruction overhead is
    # the scarce resource): ALL 6 ops ride Pool (SBUF-only, legal, and the
    # least-loaded engine); DVE keeps none of the rope.
    for i in range(len(plans)):
        x, _ = plans[i]
        dst = dsts[i]
        xlo, xhi = x[:, :, 0:HALF], x[:, :, HALF:D]
        cs, sn = cos_sb[:, :, :], sin_sb[:, :, :]
        ta = tmps[i][0][:, :, :]
        tb = tmps[i][1][:, :, :]
        tc = tmps[i][2][:, :, :]
        td = tmps[i][3][:, :, :]
        nc.gpsimd.tensor_tensor(ta, xhi, sn, AO.mult)
        nc.gpsimd.tensor_tensor(tb, xlo, cs, AO.mult)
        nc.gpsimd.tensor_tensor(dst[:, :, 0:HALF], tb, ta, AO.subtract)
        nc.gpsimd.tensor_tensor(tc, xhi, cs, AO.mult)
        nc.gpsimd.tensor_tensor(td, xlo, sn, AO.mult)
        nc.gpsimd.tensor_tensor(dst[:, :, HALF:D], tc, td, AO.add)

    return {"vaug": vaug, "kts": kts, "qts": qts, "plans": plans,
            "dsts": dsts, "ident": ident, "bias_sb": bias_sb,
            "zero_seg": zero_seg}


def _emit_body(nc, tc, pools, ext, sched, rep, pro, emit_next):
    from collections import deque

    from concourse import mybir

    f32 = mybir.dt.float32
    bf16 = mybir.dt.bfloat16
    AO = mybir.AluOpType
    AF = mybir.ActivationFunctionType
    big, ps_pool, pv_pool, pt_pool, out_pool, stat_pool = pools
    q_ext, k_ext, v_ext, cos_ext, sin_ext, out_ext = ext

    vaug = pro["vaug"]
    kts = pro["kts"]
    qts = pro["qts"]
    plans = pro["plans"]
    dsts = pro["dsts"]
    ident = pro["ident"]
    bias_sb = pro["bias_sb"]
    zero_seg = pro["zero_seg"]

    # Partial tiles whose mask is one full-tk rectangle only restrict the tq
    # range: slice the exp to [qlo,qhi) and zero the rest — no mask needed.
    def _tq_window(rects):
        if len(rects) == 1 and rects[0][0] == 0 and rects[0][1] == CK:
            return rects[0][2], rects[0][3]
        return None

    # pre-built {0,1} masks for the remaining partial tiles
    partials = [key for key in sorted(sched)
                if sched[key] is not None and _tq_window(sched[key]) is None]
    mask_tiles = {}
    if len(partials) <= MAX_PREBUILT_MASKS:
        for mi, key in enumerate(partials):
            mask_tiles[key] = _build_mask_tile(
                nc, AO, big, f"msk{mi}", sched[key], bf16)

    # ------- just-in-time strip transposes (PE, via shared ps ring) -------
    # Each strip: 4 non-accumulating transposes packed into one PSUM tile
    # (the whole-bank has_written clear only wipes bits, not data, and each
    # region is written exactly once), then ONE DVE copy lands it in SBUF.
    # Strip tiles flow through the 3-deep score ring without displacing exps.
    strip_done = set()

    def emit_strip(i, gg):
        if (i, gg) in strip_done:
            return
        strip_done.add((i, gg))
        strips = plans[i][1]
        dst = dsts[i]
        ps_t = ps_pool.tile([128, SEG], bf16, tag="ps",
                            name=f"tp{i}_{gg}_r{rep}")
        for j in range(4):
            nc.tensor.matmul(
                ps_t[:, j * 128:(j + 1) * 128],
                dst[:, gg * 4 + j, :], ident[:],
                is_transpose=True, skip_group_check=True)
        nc.vector.tensor_copy(strips[gg][:], ps_t[:])

    # ---------------- main attention loop (flat software pipeline) --------
    # 256-row q segments: only 2 PV accumulator banks, freeing PSUM for
    # [128, GRP*256] score tiles (GRP chunks share one exp instruction; each
    # QK matmul writes its own region once, so bank sharing is safe). The
    # segment stream is flattened and each group's PV matmuls are emitted one
    # step late so the scalar engine (exp, the bottleneck) never starves.
    seg_list = []
    # order follows strip availability (group-0 strips first), bridges the
    # prologue with both heads' early segments, and drains on a small one
    hs_order = [(0, 0), (0, 1), (0, 2), (0, 3), (1, 2), (1, 3),
                (0, 4), (0, 5), (0, 6), (0, 7), (1, 6), (1, 7),
                (1, 4), (1, 5), (1, 1), (1, 0)]
    if HPC != 2 or NQSEG != 8:
        hs_order = [(h, s) for h in range(HPC) for s in range(NQSEG)]
    for h, s in hs_order:
        chunks = [c for c in range(NCK) if (s, c) in sched]
        if not chunks:
            nc.sync.dma_start(
                out_ext[s * QSEG:(s + 1) * QSEG, h:h + 1, :].rearrange(
                    "(b p) h d -> p b (h d)", p=128),
                zero_seg[:])
            continue
        wins = {}
        for c in chunks:
            rects = sched[(s, c)]
            win = None if rects is None else _tq_window(rects)
            wins[c] = win if win is not None else (0, QSEG)

        def overlap(c, b, wins=wins):
            return wins[c][0] < (b + 1) * 128 and b * 128 < wins[c][1]

        seg_list.append({
            "h": h, "s": s, "chunks": chunks, "wins": wins,
            "overlap": overlap,
            "totals": {b: sum(1 for c in chunks if overlap(c, b))
                       for b in range(2)},
            "counts": {b: 0 for b in range(2)},
            "groups": [chunks[i:i + GRP]
                       for i in range(0, len(chunks), GRP)],
            "pv": None,
        })

    stream = []
    for info in seg_list:
        for ig, grp in enumerate(info["groups"]):
            stream.append((info, ig, grp))

    def emit_qk_exp(item):
        info, ig, grp = item
        h, s, wins = info["h"], info["s"], info["wins"]
        if info["pv"] is None:
            info["pv"] = [
                pv_pool.tile([128, D + 1], f32, tag=f"pv{b}",
                             name=f"pv{b}_r{rep}_{h}_{s}", bufs=1)
                for b in range(2)]
        qstrip = qts[h][s // 2]
        soff = (s % 2) * QSEG
        ps_s = ps_pool.tile([128, GRP, QSEG], f32, tag="ps",
                            name=f"ps_r{rep}_{h}_{s}_{ig}")
        for j, c in enumerate(grp):
            qlo, qhi = wins[c]
            nc.tensor.matmul(
                ps_s[:, j, qlo:qhi],
                kts[c // 4][:, (c % 4) * 128:(c % 4 + 1) * 128],
                qstrip[:, soff + qlo:soff + qhi],
                start=True, stop=True, skip_group_check=True)
        p_t = pt_pool.tile([128, GRP, QSEG], bf16, tag="pt",
                           name=f"pt_r{rep}_{h}_{s}_{ig}")
        p_flat = p_t.rearrange("p g q -> p (g q)")
        ps_flat = ps_s.rearrange("p g q -> p (g q)")
        merged = []
        for j, c in enumerate(grp):
            qlo, qhi = wins[c]
            base = j * QSEG
            if qlo > 0:
                nc.gpsimd.memset(p_flat[:, base:base + qlo], 0.0)
            if qhi < QSEG:
                nc.gpsimd.memset(p_flat[:, base + qhi:base + QSEG], 0.0)
            if merged and merged[-1][1] == base + qlo:
                merged[-1][1] = base + qhi
            else:
                merged.append([base + qlo, base + qhi])
        for a, b_ in merged:
            nc.scalar.activation(
                p_flat[:, a:b_], ps_flat[:, a:b_], AF.Exp,
                bias=bias_sb[:], scale=SCALE)
        for j, c in enumerate(grp):
            rects = sched[(s, c)]
            if rects is None or _tq_window(rects) is not None:
                continue
            if (s, c) in mask_tiles:
                m01 = mask_tiles[(s, c)]
            else:
                m01 = _build_mask_tile(nc, AO, pt_pool, "m01", rects, bf16)
            nc.vector.tensor_tensor(p_t[:, j, :], p_t[:, j, :], m01[:], AO.mult)
        return p_t

    def emit_pv(item, p_t):
        info, ig, grp = item
        for j, c in enumerate(grp):
            for b in range(2):
                if not info["overlap"](c, b):
                    continue
                st = info["counts"][b] == 0
                sp = info["counts"][b] == info["totals"][b] - 1
                info["counts"][b] += 1
                nc.tensor.matmul(
                    info["pv"][b][:],
                    p_t[:, j, b * 128:(b + 1) * 128],
                    vaug[:, c, :], start=st, stop=sp)
        if ig == len(info["groups"]) - 1:
            emit_epilogue(info)

    epi_count = [0]

    def emit_epilogue(info):
        # stage the whole 256-row segment in one tile -> ONE output DMA
        # (bf16: halves the output writeback; host upcasts to f32)
        h, s = info["h"], info["s"]
        o_seg = out_pool.tile([128, 2, D], bf16, tag="o",
                              name=f"o_r{rep}_{h}_{s}")
        for b in range(2):
            if info["totals"][b] == 0:
                nc.vector.memset(o_seg[:, b, :], 0.0)
                continue
            # covered rows have >=256 exp terms (each > e^-9), so rowsum is
            # bounded away from 0 -- reciprocal directly from PSUM, no
            # epsilon clamp (DVE per-instruction overhead is the scarce
            # resource, not precision)
            src = info["pv"][b]
            rec = stat_pool.tile([128, 1], f32, tag="rec",
                                 name=f"rec_r{rep}_{h}_{s}_{b}")
            nc.vector.reciprocal(rec[:], src[:, D:D + 1])
            # every 8th normalization mul rides ACT (Copy-with-scale; Copy
            # shares the exp table, so no table reload): with the whole rope
            # on Pool, DVE and ACT balance at ~4 ACT muls
            if epi_count[0] % 8 == 0:
                nc.scalar.activation(o_seg[:, b, :], src[:, 0:D], AF.Copy,
                                     bias=0.0, scale=rec[:])
            else:
                nc.vector.tensor_scalar_mul(
                    o_seg[:, b, :], src[:, 0:D], rec[:])
            epi_count[0] += 1
        # out rides SP's HWDGE: gpsimd DMA is software-DGE (~us per trigger,
        # one queue) and ACT is the exp bottleneck
        nc.sync.dma_start(
            out_ext[s * QSEG:(s + 1) * QSEG, h:h + 1, :].rearrange(
                "(b p) h d -> p b (h d)", p=128),
            o_seg[:])

    # Strip first-use schedule: strips are emitted at their first-use
    # position (mandatory), plus at most one prefetch per strip-free
    # position so back-to-back strip needs never bunch up in the ring.
    first_use = {}
    for idx, (info, ig, grp) in enumerate(stream):
        for c in grp:
            first_use.setdefault((0, c // 4), idx)
        first_use.setdefault((1 + info["h"], info["s"] // 2), idx)
    by_pos = {}
    for key, idx in first_use.items():
        by_pos.setdefault(idx, []).append(key)

    # PV rides TWO groups behind QK/exp: PE gets a full group of lookahead,
    # so segment-boundary epilogue latency (freeing the single-buffered pv
    # banks) never backs up into the exp stream.
    pending = deque()
    # 45%: early enough that Pool's hoisted rope block (now all 18 rope ops)
    # finishes under this rep's tail, late enough that this rep's own rope
    # reads of k_sb/q_sb are long done before the next rep's loads land
    hoist_at = (len(stream) * 9) // 20
    for idx, item in enumerate(stream):
        if emit_next is not None and idx == hoist_at:
            # software-pipeline across reps: emit the NEXT rep's prologue
            # (loads + rope + vaug) here, so in-order engines run it under
            # this rep's tail instead of serializing it at the boundary
            emit_next()
        must = by_pos.get(idx, [])
        for key in must:
            emit_strip(*key)
        if not must:
            for fidx in (idx + 1, idx + 2):
                nxt = [k for k in by_pos.get(fidx, [])
                       if k not in strip_done]
                if nxt:
                    emit_strip(*nxt[0])
                    break
        p_t = emit_qk_exp(item)
        pending.append((item, p_t))
        if len(pending) > 2:
            emit_pv(*pending.popleft())
    while pending:
        emit_pv(*pending.popleft())


def _build_nc(sched, reps=1):
    from contextlib import ExitStack

    from concourse import bacc, mybir, tile

    f32 = mybir.dt.float32

    nc = bacc.Bacc(None, target_bir_lowering=False)
    bf16 = mybir.dt.bfloat16
    q_ext = nc.declare_dram_parameter("q", [T, HPC, D], bf16, isOutput=False)
    k_ext = nc.declare_dram_parameter("k", [T, 1, D], bf16, isOutput=False)
    v_ext = nc.declare_dram_parameter("v", [T, 1, D], bf16, isOutput=False)
    cos_ext = nc.declare_dram_parameter("cos", [T, HALF], bf16, isOutput=False)
    sin_ext = nc.declare_dram_parameter("sin", [T, HALF], bf16, isOutput=False)
    out_ext = nc.declare_dram_parameter("out", [T, HPC, D], bf16, isOutput=True)
    ext = (q_ext, k_ext, v_ext, cos_ext, sin_ext, out_ext)

    with tile.TileContext(nc) as tc, ExitStack() as ctx:
        big = ctx.enter_context(tc.tile_pool(name="big", bufs=1))
        ps_pool = ctx.enter_context(
            tc.tile_pool(name="psum", bufs=3, space="PSUM"))
        pv_pool = ctx.enter_context(
            tc.tile_pool(name="pvp", bufs=1, space="PSUM"))
        pt_pool = ctx.enter_context(tc.tile_pool(name="ptsb", bufs=6))
        out_pool = ctx.enter_context(tc.tile_pool(name="outp", bufs=6))
        stat_pool = ctx.enter_context(tc.tile_pool(name="stat", bufs=16))
        pools = (big, ps_pool, pv_pool, pt_pool, out_pool, stat_pool)
        pro = _emit_prologue(nc, pools, ext, 0)
        for rep in range(reps):
            holder = {}
            if rep + 1 < reps:
                def emit_next(r=rep, h=holder):
                    if "pro" not in h:
                        h["pro"] = _emit_prologue(nc, pools, ext, r + 1)
            else:
                emit_next = None
            _emit_body(nc, tc, pools, ext, sched, rep, pro, emit_next)
            if rep + 1 < reps:
                pro = holder.get("pro")
                if pro is None:
                    pro = _emit_prologue(nc, pools, ext, rep + 1)
    nc.compile()
    return nc


def _shards(q, k, v, cos, sin):
    # inputs staged bf16: halves HBM traffic (the per-rep bottleneck); all
    # arithmetic (rope, matmuls, softmax) still runs on device
    import ml_dtypes
    bf = ml_dtypes.bfloat16
    cos_b = np.ascontiguousarray(cos, dtype=bf)
    sin_b = np.ascontiguousarray(sin, dtype=bf)
    in_maps = []
    for c in range(N_CORES):
        kv = c // 2
        in_maps.append({
            "q": np.ascontiguousarray(q[:, 2 * c:2 * c + 2, :]).astype(bf),
            "k": np.ascontiguousarray(k[:, kv:kv + 1, :]).astype(bf),
            "v": np.ascontiguousarray(v[:, kv:kv + 1, :]).astype(bf),
            "cos": cos_b,
            "sin": sin_b,
        })
    return in_maps


def kernel(q, k, v, cos, sin, q_ranges, k_ranges):
    global LAST_EXEC_NS, LAST_RESULT
    from concourse.bass_utils import run_bass_kernel_spmd

    sched = _build_schedule(q_ranges, k_ranges)
    key = _sched_key(sched)
    if key not in _NEFF_CACHE:
        _NEFF_CACHE[key] = _build_nc(sched)
    nc = _NEFF_CACHE[key]

    res = run_bass_kernel_spmd(
        nc, _shards(q, k, v, cos, sin), core_ids=list(range(N_CORES)),
        trace=PROFILE)
    LAST_RESULT = res
    LAST_EXEC_NS = getattr(res, "exec_time_ns", None)
    out = np.concatenate(
        [res.results[c]["out"].reshape(T, HPC, D) for c in range(N_CORES)],
        axis=1)
    return out.astype(np.float32, copy=False)



# revision 60
# speedup vs baseline: 1.2291x; 1.2291x over previous
"""Sparse (range-masked) GQA attention on 8 Trainium2 NeuronCores.

Strategy: tensor-parallel over heads. Core c owns q heads {2c, 2c+1} and kv
head c//2, so each core runs an independent single-core attention over the
full sequence — no collectives; the host concatenates the 8 head-slices.

Per-core kernel (Bass/Tile), HW-calibrated design points:
  1. Inputs staged bf16 on the host (halves HBM traffic; arithmetic stays on
     device); all loads ride SP's HWDGE queue (gpsimd DMA is software-DGE,
     ~us/trigger), halved and ordered k/cos/sin -> q -> v per half.
  2. RoPE + K^T/Q^T strip transposes are emitted JUST-IN-TIME per 512-token
     strip, right before the first q-segment that reads the strip: RoPE (all
     on DVE, bf16 2x modes), 4 packed PE transposes into a ps-ring PSUM
     tile, one DVE copy to SBUF. Engines execute in order, so a monolithic
     prologue would queue the next rep's rope behind this rep's tail; JIT
     emission shrinks the rep boundary to one strip chain. Strips, rope
     outputs and V are parity-buffered across reps.
  3. Flat software pipeline over (head, 256-row q segment, group of GRP=4
     active 128-wide k chunks):
       S^T[tk,tq] = KT_chunk.T @ QT_seg          (PE, bf16, PSUM fp32,
                                                  3-deep 2-bank ring)
       P^T = Exp(scale*S^T - 4)                  (ACT, one [128,<=1024] instr
                                                  per group; ACT is the
                                                  bottleneck: ~0.35ns/col +
                                                  ~350ns/instr on HW)
       irregular partial tiles only: P^T *= mask01 (affine_select-built masks)
       out[tq,0:129] += P^T_block.T @ [V | 1]    (PE; ones column => row sums)
     PV matmuls are emitted TWO groups late so PE lookahead absorbs the
     segment-boundary epilogue latency (single-buffered PV banks).
  4. out = PV / max(rowsum, tiny) staged bf16 -> one SP DMA per segment;
     host upcasts to fp32. Uncovered q rows come out as exactly 0.

The tile schedule (skip / dense / partial+rectangles) is computed on the host
from q_ranges/k_ranges — scheduling metadata only; all arithmetic on q/k/v
runs on device. The compiled NEFF is cached per schedule signature.
"""

import math

import numpy as np

T, HQ, HKV, D = 2048, 16, 4, 128
N_CORES = 8
HPC = HQ // N_CORES          # q heads per core
SEG = 512                    # strip width (transposed Q/K layout granularity)
QSEG = 256                   # main-loop q segment: 2 PV banks -> wider exps
GRP = 4                      # k chunks per PSUM score tile / exp instruction
CK = 128                     # tk chunk (contraction tile)
NSEG = T // SEG
NQSEG = T // QSEG
NCK = T // CK
NT = T // 128                # 128-row t-tiles
HALF = D // 2
SCALE = 1.0 / math.sqrt(D)
EXP_BIAS = -4.0              # constant shift; cancels in softmax normalization
SUM_EPS = 1e-30
MAX_PREBUILT_MASKS = 48      # SBUF budget cap; beyond this, build inline

# "pe": PE transpose + ACT/DVE copies (fastest measured — each crossbar
# transpose costs ~2us of DMA-ucode overhead, so "dma"/"sbuf" lose badly);
# "dma": DRAM scratch + crossbar;  "sbuf": direct SBUF->SBUF crossbar
TRANSPOSE_MODE = "pe"

PROFILE = False
LAST_EXEC_NS = None
LAST_RESULT = None

_NEFF_CACHE = {}


def _build_schedule(q_ranges, k_ranges):
    """Per (qseg, kchunk) tile: absent=skip, None=dense, list=mask rectangles.

    Rectangles are in device-tile coordinates: (klo, khi) along the partition
    (tk) axis, (qlo, qhi) along the free (tq) axis, clipped to the tile.
    """
    q_ranges = np.asarray(q_ranges, dtype=np.int64)
    k_ranges = np.asarray(k_ranges, dtype=np.int64)
    idx = np.arange(T)
    qm = (idx[None, :] >= q_ranges[:, :1]) & (idx[None, :] < q_ranges[:, 1:])
    km = (idx[None, :] >= k_ranges[:, :1]) & (idx[None, :] < k_ranges[:, 1:])
    mask = np.zeros((T, T), dtype=bool)
    for r in range(len(q_ranges)):
        if qm[r].any() and km[r].any():
            mask[np.ix_(qm[r], km[r])] = True
    sched = {}
    for s in range(NQSEG):
        for c in range(NCK):
            sub = mask[s * QSEG:(s + 1) * QSEG, c * CK:(c + 1) * CK]
            if not sub.any():
                continue
            if sub.all():
                sched[(s, c)] = None
                continue
            rects = []
            for r in range(len(q_ranges)):
                qlo = max(int(q_ranges[r, 0]) - s * QSEG, 0)
                qhi = min(int(q_ranges[r, 1]) - s * QSEG, QSEG)
                klo = max(int(k_ranges[r, 0]) - c * CK, 0)
                khi = min(int(k_ranges[r, 1]) - c * CK, CK)
                if qlo < qhi and klo < khi:
                    rects.append((klo, khi, qlo, qhi))
            sched[(s, c)] = rects
    return sched


def _sched_key(sched):
    return tuple(sorted(
        (k, -1) if v is None else (k, tuple(v)) for k, v in sched.items()
    ))


def _rect_mask(nc, AO, m, rect, first):
    """AND-chain of affine_select half-planes building a rect indicator."""
    klo, khi, qlo, qhi = rect
    if first:
        nc.gpsimd.memset(m, 1.0)
    if klo > 0:
        nc.gpsimd.affine_select(
            out=m, in_=m, compare_op=AO.is_ge, fill=0.0,
            base=-klo, pattern=[[0, QSEG]], channel_multiplier=1)
    if khi < CK:
        # walrus codegen lacks is_lt: p < khi  <=>  khi - p > 0
        nc.gpsimd.affine_select(
            out=m, in_=m, compare_op=AO.is_gt, fill=0.0,
            base=khi, pattern=[[0, QSEG]], channel_multiplier=-1)
    if qlo > 0:
        nc.gpsimd.affine_select(
            out=m, in_=m, compare_op=AO.is_ge, fill=0.0,
            base=-qlo, pattern=[[1, QSEG]], channel_multiplier=0)
    if qhi < QSEG:
        nc.gpsimd.affine_select(
            out=m, in_=m, compare_op=AO.is_gt, fill=0.0,
            base=qhi, pattern=[[-1, QSEG]], channel_multiplier=0)


def _build_mask_tile(nc, AO, pool, m01_tag, rects, bf16):
    m01 = pool.tile([128, QSEG], bf16, tag=m01_tag, name=m01_tag)
    if len(rects) == 1:
        _rect_mask(nc, AO, m01[:], rects[0], first=True)
    else:
        acc = pool.tile([128, QSEG], bf16, tag=m01_tag + "a", name=m01_tag + "a")
        _rect_mask(nc, AO, m01[:], rects[0], first=True)
        for rect in rects[1:]:
            _rect_mask(nc, AO, acc[:], rect, first=True)
            nc.vector.tensor_tensor(m01[:], m01[:], acc[:], AO.max)
    return m01


def _emit_prologue(nc, pools, ext, rep):
    """Loads + constants + vaug + RoPE for one rep. For rep r+1 this is
    invoked MID-way through rep r's stream, so in-order engines execute it
    under rep r's tail instead of serializing it at the rep boundary."""
    from concourse import mybir

    f32 = mybir.dt.float32
    bf16 = mybir.dt.bfloat16
    AO = mybir.AluOpType
    big = pools[0]
    q_ext, k_ext, v_ext, cos_ext, sin_ext, out_ext = ext

    def btile(shape, dtype, tag):
        return big.tile(shape, dtype, tag=tag, name=f"{tag}_r{rep}")

    # -------- loads (k/cos/sin first, halved, so RoPE starts early) --------
    # ALL inputs ride the sync (SP HWDGE) queue.
    GT = NT // 2          # t-tiles per DMA half
    cos_sb = btile([128, NT, HALF], bf16, "cos_sb")
    sin_sb = btile([128, NT, HALF], bf16, "sin_sb")
    k_sb = btile([128, NT, D], bf16, "k_sb")
    q_sb = btile([128, NT, HPC, D], bf16, "q_sb")
    v_sb = btile([128, NT, D], bf16, "v_sb")
    def load_half(g):
        a, b = g * GT, (g + 1) * GT
        tt = slice(a * 128, b * 128)
        nt = slice(a, b)
        nc.sync.dma_start(
            k_sb[:, nt, :],
            k_ext[tt, :, :].rearrange("(n p) h d -> p n (h d)", p=128))
        nc.sync.dma_start(
            cos_sb[:, nt, :],
            cos_ext[tt, :].rearrange("(n p) d -> p n d", p=128))
        nc.sync.dma_start(
            sin_sb[:, nt, :],
            sin_ext[tt, :].rearrange("(n p) d -> p n d", p=128))
        # q both heads at once: contiguous 1 KiB rows per descriptor
        nc.sync.dma_start(
            q_sb[:, nt, :, :],
            q_ext[tt, :, :].rearrange("(n p) h d -> p n h d", p=128))
        nc.sync.dma_start(
            v_sb[:, nt, :],
            v_ext[tt, :, :].rearrange("(n p) h d -> p n (h d)", p=128))

    load_half(0)
    load_half(1)

    # constants (same values every rep; rebuilds are dep-tracked and cheap)
    bias_sb = btile([128, 1], f32, "bias")
    nc.vector.memset(bias_sb[:], EXP_BIAS)
    zero_seg = btile([128, 2, D], bf16, "zero")
    nc.vector.memset(zero_seg[:], 0.0)

    # V with an appended ones column: [tk, (V | 1)] in bf16. Parity-buffered
    # across reps (and copied in halves) so this rep's copy never waits on
    # the previous rep's last PV reads.
    par = rep % 2
    vaug = btile([128, NT, D + 1], bf16, f"vaug_p{par}")
    nc.gpsimd.memset(vaug[:, :, D:D + 1], 1.0)
    nc.gpsimd.tensor_copy(vaug[:, 0:GT, 0:D], v_sb[:, 0:GT, :])
    nc.gpsimd.tensor_copy(vaug[:, GT:NT, 0:D], v_sb[:, GT:NT, :])

    # strips / rope outputs parity-buffered across reps: this rep's writes
    # never WAR against the previous rep's main-loop reads
    kts = [btile([128, SEG], bf16, f"kts{g}_p{par}") for g in range(NSEG)]
    qts = [[btile([128, SEG], bf16, f"qts{h}_{g}_p{par}") for g in range(NSEG)]
           for h in range(HPC)]
    plans = [(k_sb[:, :, :], kts)] + [
        (q_sb[:, :, h, :], qts[h]) for h in range(HPC)]
    dsts = [btile([128, NT, D], bf16, f"rope{i}_p{par}")
            for i in range(len(plans))]
    tmps = [[btile([128, NT, HALF], bf16, f"t{j}_{i}") for j in range(4)]
            for i in range(len(plans))]
    ident = btile([128, 128], bf16, "ident")
    nc.gpsimd.memset(ident[:], 0.0)
    nc.gpsimd.affine_select(
        out=ident[:], in_=ident[:], compare_op=AO.not_equal, fill=1.0,
        base=0, pattern=[[-1, 128]], channel_multiplier=1)

    # RoPE, whole-tensor (fewest/widest ops -- per-instruction overhead is
    # the scarce resource): the 4 products ride Pool (SBUF-only, legal, and
    # otherwise idle), the 2 combines ride DVE (the busier engine keeps only
    # what it must).
    for i in range(len(plans)):
        x, _ = plans[i]
        dst = dsts[i]
        xlo, xhi = x[:, :, 0:HALF], x[:, :, HALF:D]
        cs, sn = cos_sb[:, :, :], sin_sb[:, :, :]
        ta = tmps[i][0][:, :, :]
        tb = tmps[i][1][:, :, :]
        tc = tmps[i][2][:, :, :]
        td = tmps[i][3][:, :, :]
        nc.gpsimd.tensor_tensor(ta, xhi, sn, AO.mult)
        nc.gpsimd.tensor_tensor(tb, xlo, cs, AO.mult)
        nc.vector.tensor_tensor(dst[:, :, 0:HALF], tb, ta, AO.subtract)
        nc.gpsimd.tensor_tensor(tc, xhi, cs, AO.mult)
        nc.gpsimd.tensor_tensor(td, xlo, sn, AO.mult)
        nc.vector.tensor_tensor(dst[:, :, HALF:D], tc, td, AO.add)

    return {"vaug": vaug, "kts": kts, "qts": qts, "plans": plans,
            "dsts": dsts, "ident": ident, "bias_sb": bias_sb,
            "zero_seg": zero_seg}


def _emit_body(nc, tc, pools, ext, sched, rep, pro, emit_next):
    from collections import deque

    from concourse import mybir

    f32 = mybir.dt.float32
    bf16 = mybir.dt.bfloat16
    AO = mybir.AluOpType
    AF = mybir.ActivationFunctionType
    big, ps_pool, pv_pool, pt_pool, out_pool, stat_pool = pools
    q_ext, k_ext, v_ext, cos_ext, sin_ext, out_ext = ext

    vaug = pro["vaug"]
    kts = pro["kts"]
    qts = pro["qts"]
    plans = pro["plans"]
    dsts = pro["dsts"]
    ident = pro["ident"]
    bias_sb = pro["bias_sb"]
    zero_seg = pro["zero_seg"]

    # Partial tiles whose mask is one full-tk rectangle only restrict the tq
    # range: slice the exp to [qlo,qhi) and zero the rest — no mask needed.
    def _tq_window(rects):
        if len(rects) == 1 and rects[0][0] == 0 and rects[0][1] == CK:
            return rects[0][2], rects[0][3]
        return None

    # pre-built {0,1} masks for the remaining partial tiles
    partials = [key for key in sorted(sched)
                if sched[key] is not None and _tq_window(sched[key]) is None]
    mask_tiles = {}
    if len(partials) <= MAX_PREBUILT_MASKS:
        for mi, key in enumerate(partials):
            mask_tiles[key] = _build_mask_tile(
                nc, AO, big, f"msk{mi}", sched[key], bf16)

    # ------- just-in-time strip transposes (PE, via shared ps ring) -------
    # Each strip: 4 non-accumulating transposes packed into one PSUM tile
    # (the whole-bank has_written clear only wipes bits, not data, and each
    # region is written exactly once), then ONE DVE copy lands it in SBUF.
    # Strip tiles flow through the 3-deep score ring without displacing exps.
    strip_done = set()

    def emit_strip(i, gg):
        if (i, gg) in strip_done:
            return
        strip_done.add((i, gg))
        strips = plans[i][1]
        dst = dsts[i]
        ps_t = ps_pool.tile([128, SEG], bf16, tag="ps",
                            name=f"tp{i}_{gg}_r{rep}")
        for j in range(4):
            nc.tensor.matmul(
                ps_t[:, j * 128:(j + 1) * 128],
                dst[:, gg * 4 + j, :], ident[:],
                is_transpose=True, skip_group_check=True)
        nc.vector.tensor_copy(strips[gg][:], ps_t[:])

    # ---------------- main attention loop (flat software pipeline) --------
    # 256-row q segments: only 2 PV accumulator banks, freeing PSUM for
    # [128, GRP*256] score tiles (GRP chunks share one exp instruction; each
    # QK matmul writes its own region once, so bank sharing is safe). The
    # segment stream is flattened and each group's PV matmuls are emitted one
    # step late so the scalar engine (exp, the bottleneck) never starves.
    seg_list = []
    # order follows strip availability (group-0 strips first), bridges the
    # prologue with both heads' early segments, and drains on a small one
    hs_order = [(0, 0), (0, 1), (0, 2), (0, 3), (1, 2), (1, 3),
                (0, 4), (0, 5), (0, 6), (0, 7), (1, 6), (1, 7),
                (1, 4), (1, 5), (1, 1), (1, 0)]
    if HPC != 2 or NQSEG != 8:
        hs_order = [(h, s) for h in range(HPC) for s in range(NQSEG)]
    for h, s in hs_order:
        chunks = [c for c in range(NCK) if (s, c) in sched]
        if not chunks:
            nc.sync.dma_start(
                out_ext[s * QSEG:(s + 1) * QSEG, h:h + 1, :].rearrange(
                    "(b p) h d -> p b (h d)", p=128),
                zero_seg[:])
            continue
        wins = {}
        for c in chunks:
            rects = sched[(s, c)]
            win = None if rects is None else _tq_window(rects)
            wins[c] = win if win is not None else (0, QSEG)

        def overlap(c, b, wins=wins):
            return wins[c][0] < (b + 1) * 128 and b * 128 < wins[c][1]

        seg_list.append({
            "h": h, "s": s, "chunks": chunks, "wins": wins,
            "overlap": overlap,
            "totals": {b: sum(1 for c in chunks if overlap(c, b))
                       for b in range(2)},
            "counts": {b: 0 for b in range(2)},
            "groups": [chunks[i:i + GRP]
                       for i in range(0, len(chunks), GRP)],
            "pv": None,
        })

    stream = []
    for info in seg_list:
        for ig, grp in enumerate(info["groups"]):
            stream.append((info, ig, grp))

    def emit_qk_exp(item):
        info, ig, grp = item
        h, s, wins = info["h"], info["s"], info["wins"]
        if info["pv"] is None:
            info["pv"] = [
                pv_pool.tile([128, D + 1], f32, tag=f"pv{b}",
                             name=f"pv{b}_r{rep}_{h}_{s}", bufs=1)
                for b in range(2)]
        qstrip = qts[h][s // 2]
        soff = (s % 2) * QSEG
        ps_s = ps_pool.tile([128, GRP, QSEG], f32, tag="ps",
                            name=f"ps_r{rep}_{h}_{s}_{ig}")
        for j, c in enumerate(grp):
            qlo, qhi = wins[c]
            nc.tensor.matmul(
                ps_s[:, j, qlo:qhi],
                kts[c // 4][:, (c % 4) * 128:(c % 4 + 1) * 128],
                qstrip[:, soff + qlo:soff + qhi],
                start=True, stop=True, skip_group_check=True)
        p_t = pt_pool.tile([128, GRP, QSEG], bf16, tag="pt",
                           name=f"pt_r{rep}_{h}_{s}_{ig}")
        p_flat = p_t.rearrange("p g q -> p (g q)")
        ps_flat = ps_s.rearrange("p g q -> p (g q)")
        merged = []
        for j, c in enumerate(grp):
            qlo, qhi = wins[c]
            base = j * QSEG
            if qlo > 0:
                nc.gpsimd.memset(p_flat[:, base:base + qlo], 0.0)
            if qhi < QSEG:
                nc.gpsimd.memset(p_flat[:, base + qhi:base + QSEG], 0.0)
            if merged and merged[-1][1] == base + qlo:
                merged[-1][1] = base + qhi
            else:
                merged.append([base + qlo, base + qhi])
        for a, b_ in merged:
            nc.scalar.activation(
                p_flat[:, a:b_], ps_flat[:, a:b_], AF.Exp,
                bias=bias_sb[:], scale=SCALE)
        for j, c in enumerate(grp):
            rects = sched[(s, c)]
            if rects is None or _tq_window(rects) is not None:
                continue
            if (s, c) in mask_tiles:
                m01 = mask_tiles[(s, c)]
            else:
                m01 = _build_mask_tile(nc, AO, pt_pool, "m01", rects, bf16)
            nc.vector.tensor_tensor(p_t[:, j, :], p_t[:, j, :], m01[:], AO.mult)
        return p_t

    def emit_pv(item, p_t):
        info, ig, grp = item
        for j, c in enumerate(grp):
            for b in range(2):
                if not info["overlap"](c, b):
                    continue
                st = info["counts"][b] == 0
                sp = info["counts"][b] == info["totals"][b] - 1
                info["counts"][b] += 1
                nc.tensor.matmul(
                    info["pv"][b][:],
                    p_t[:, j, b * 128:(b + 1) * 128],
                    vaug[:, c, :], start=st, stop=sp)
        if ig == len(info["groups"]) - 1:
            emit_epilogue(info)

    epi_count = [0]

    def emit_epilogue(info):
        # stage the whole 256-row segment in one tile -> ONE output DMA
        # (bf16: halves the output writeback; host upcasts to f32)
        h, s = info["h"], info["s"]
        o_seg = out_pool.tile([128, 2, D], bf16, tag="o",
                              name=f"o_r{rep}_{h}_{s}")
        for b in range(2):
            if info["totals"][b] == 0:
                nc.vector.memset(o_seg[:, b, :], 0.0)
                continue
            # covered rows have >=256 exp terms (each > e^-9), so rowsum is
            # bounded away from 0 -- reciprocal directly from PSUM, no
            # epsilon clamp (DVE per-instruction overhead is the scarce
            # resource, not precision)
            src = info["pv"][b]
            rec = stat_pool.tile([128, 1], f32, tag="rec",
                                 name=f"rec_r{rep}_{h}_{s}_{b}")
            nc.vector.reciprocal(rec[:], src[:, D:D + 1])
            # every 4th normalization mul rides ACT (Copy-with-scale; Copy
            # shares the exp table, so no table reload) to shave the
            # DVE-bound epilogue stream
            if epi_count[0] % 4 == 0:
                nc.scalar.activation(o_seg[:, b, :], src[:, 0:D], AF.Copy,
                                     bias=0.0, scale=rec[:])
            else:
                nc.vector.tensor_scalar_mul(
                    o_seg[:, b, :], src[:, 0:D], rec[:])
            epi_count[0] += 1
        # out rides SP's HWDGE: gpsimd DMA is software-DGE (~us per trigger,
        # one queue) and ACT is the exp bottleneck
        nc.sync.dma_start(
            out_ext[s * QSEG:(s + 1) * QSEG, h:h + 1, :].rearrange(
                "(b p) h d -> p b (h d)", p=128),
            o_seg[:])

    # Strip first-use schedule: strips are emitted at their first-use
    # position (mandatory), plus at most one prefetch per strip-free
    # position so back-to-back strip needs never bunch up in the ring.
    first_use = {}
    for idx, (info, ig, grp) in enumerate(stream):
        for c in grp:
            first_use.setdefault((0, c // 4), idx)
        first_use.setdefault((1 + info["h"], info["s"] // 2), idx)
    by_pos = {}
    for key, idx in first_use.items():
        by_pos.setdefault(idx, []).append(key)

    # PV rides TWO groups behind QK/exp: PE gets a full group of lookahead,
    # so segment-boundary epilogue latency (freeing the single-buffered pv
    # banks) never backs up into the exp stream.
    pending = deque()
    # 50%: early enough that Pool's hoisted rope-product block (13 us,
    # slower than modeled) drains before the boundary; this rep's own rope
    # reads of k_sb/q_sb are long done, so next-rep loads landing here are
    # WAR-safe
    hoist_at = len(stream) // 2
    for idx, item in enumerate(stream):
        if emit_next is not None and idx == hoist_at:
            # software-pipeline across reps: emit the NEXT rep's prologue
            # (loads + rope + vaug) here, so in-order engines run it under
            # this rep's tail instead of serializing it at the boundary
            emit_next()
        must = by_pos.get(idx, [])
        for key in must:
            emit_strip(*key)
        if not must:
            for fidx in (idx + 1, idx + 2):
                nxt = [k for k in by_pos.get(fidx, [])
                       if k not in strip_done]
                if nxt:
                    emit_strip(*nxt[0])
                    break
        p_t = emit_qk_exp(item)
        pending.append((item, p_t))
        if len(pending) > 2:
            emit_pv(*pending.popleft())
    while pending:
        emit_pv(*pending.popleft())


def _build_nc(sched, reps=1):
    from contextlib import ExitStack

    from concourse import bacc, mybir, tile

    f32 = mybir.dt.float32

    nc = bacc.Bacc(None, target_bir_lowering=False)
    bf16 = mybir.dt.bfloat16
    q_ext = nc.declare_dram_parameter("q", [T, HPC, D], bf16, isOutput=False)
    k_ext = nc.declare_dram_parameter("k", [T, 1, D], bf16, isOutput=False)
    v_ext = nc.declare_dram_parameter("v", [T, 1, D], bf16, isOutput=False)
    cos_ext = nc.declare_dram_parameter("cos", [T, HALF], bf16, isOutput=False)
    sin_ext = nc.declare_dram_parameter("sin", [T, HALF], bf16, isOutput=False)
    out_ext = nc.declare_dram_parameter("out", [T, HPC, D], bf16, isOutput=True)
    ext = (q_ext, k_ext, v_ext, cos_ext, sin_ext, out_ext)

    with tile.TileContext(nc) as tc, ExitStack() as ctx:
        big = ctx.enter_context(tc.tile_pool(name="big", bufs=1))
        ps_pool = ctx.enter_context(
            tc.tile_pool(name="psum", bufs=3, space="PSUM"))
        pv_pool = ctx.enter_context(
            tc.tile_pool(name="pvp", bufs=1, space="PSUM"))
        pt_pool = ctx.enter_context(tc.tile_pool(name="ptsb", bufs=6))
        out_pool = ctx.enter_context(tc.tile_pool(name="outp", bufs=6))
        stat_pool = ctx.enter_context(tc.tile_pool(name="stat", bufs=16))
        pools = (big, ps_pool, pv_pool, pt_pool, out_pool, stat_pool)
        pro = _emit_prologue(nc, pools, ext, 0)
        for rep in range(reps):
            holder = {}
            if rep + 1 < reps:
                def emit_next(r=rep, h=holder):
                    if "pro" not in h:
                        h["pro"] = _emit_prologue(nc, pools, ext, r + 1)
            else:
                emit_next = None
            _emit_body(nc, tc, pools, ext, sched, rep, pro, emit_next)
            if rep + 1 < reps:
                pro = holder.get("pro")
                if pro is None:
                    pro = _emit_prologue(nc, pools, ext, rep + 1)
    nc.compile()
    return nc


def _shards(q, k, v, cos, sin):
    # inputs staged bf16: halves HBM traffic (the per-rep bottleneck); all
    # arithmetic (rope, matmuls, softmax) still runs on device
    import ml_dtypes
    bf = ml_dtypes.bfloat16
    cos_b = np.ascontiguousarray(cos, dtype=bf)
    sin_b = np.ascontiguousarray(sin, dtype=bf)
    in_maps = []
    for c in range(N_CORES):
        kv = c // 2
        in_maps.append({
            "q": np.ascontiguousarray(q[:, 2 * c:2 * c + 2, :]).astype(bf),
            "k": np.ascontiguousarray(k[:, kv:kv + 1, :]).astype(bf),
            "v": np.ascontiguousarray(v[:, kv:kv + 1, :]).astype(bf),
            "cos": cos_b,
            "sin": sin_b,
        })
    return in_maps


def kernel(q, k, v, cos, sin, q_ranges, k_ranges):
    global LAST_EXEC_NS, LAST_RESULT
    from concourse.bass_utils import run_bass_kernel_spmd

    sched = _build_schedule(q_ranges, k_ranges)
    key = _sched_key(sched)
    if key not in _NEFF_CACHE:
        _NEFF_CACHE[key] = _build_nc(sched)
    nc = _NEFF_CACHE[key]

    res = run_bass_kernel_spmd(
        nc, _shards(q, k, v, cos, sin), core_ids=list(range(N_CORES)),
        trace=PROFILE)
    LAST_RESULT = res
    LAST_EXEC_NS = getattr(res, "exec_time_ns", None)
    out = np.concatenate(
        [res.results[c]["out"].reshape(T, HPC, D) for c in range(N_CORES)],
        axis=1)
    return out.astype(np.float32, copy=False)

